# revision 3
# baseline (speedup 1.0000x reference)
"""HAGMoE Trainium2 kernel: hierarchical-routed 24-expert MoE, expert-parallel on 8 cores.

Reference computation (B=1024, H=768, I=3072, G=3 groups, E=8 experts/group):
    h_cond  = cat(h_fused, h_aspect) @ Wc + bc
    p_group = softmax(h_fused @ Wg + bg)
    p_exp   = softmax(h_cond @ Wr[g] + br[g])  per group
    h_moe   = sum_{g,e} p_group[:,g] * p_exp[:,g,e] * fc2(gelu(fc1(h_fused)))
    out     = h_fused + h_moe
Sharding: core c owns experts (g, c) for g=0..2 (one expert per group).  The
cond-proj is folded through the expert routers on the host (Wcr = Wc @ Wr), and
within-group expert columns are permuted per core so every core's experts sit at
logit columns {0, 8, 16} -> identical SPMD program, per-core weight data only.
Expert matmuls run in fp8e4 DoubleRow mode (2x PE throughput, fp32 PSUM
accumulate) with host-side weight scaling to dodge fp8 subnormals; routing
stays bf16/fp32.  The tensor queue runs fc1(expert0) first so the PE starts as
soon as x8+W1[0] land; routing slots in behind it.  Host gathers:
out = h_fused + sum_c partial_c.
"""

import os
import sys

if "/opt/trn_rl_repo" not in sys.path:
    sys.path.insert(0, "/opt/trn_rl_repo")

import numpy as np
import ml_dtypes

B, H, I, G, E = 1024, 768, 3072, 3, 8
NCORES = 8
BF16 = ml_dtypes.bfloat16
FP8 = ml_dtypes.float8_e4m3

SX = 16.0   # h_fused scale before fp8 cast
SW1 = 64.0  # W1 scale before fp8 cast
SW2 = 64.0  # W2 scale before fp8 cast

_nc_cache = None


def _build_nc():
    from concourse import bacc
    import concourse.mybir as mybir
    from concourse.tile import TileContext

    dt = mybir.dt
    AF = mybir.ActivationFunctionType
    DR = mybir.MatmulPerfMode.DoubleRow

    nc = bacc.Bacc("TRN2", target_bir_lowering=False, debug=False, num_devices=NCORES)

    # ---- DRAM I/O ----
    xtf16_d = nc.dram_tensor("xtf16", [H, B], dt.bfloat16, kind="ExternalInput")
    xta16_d = nc.dram_tensor("xta16", [H, B], dt.bfloat16, kind="ExternalInput")
    xt8_d = nc.dram_tensor("xt8", [H, B], dt.float8e4, kind="ExternalInput")
    rf_d = nc.dram_tensor("rf", [H, 27], dt.bfloat16, kind="ExternalInput")
    ra_d = nc.dram_tensor("ra", [H, 27], dt.bfloat16, kind="ExternalInput")
    bcat_d = nc.dram_tensor("bcat", [27, 1], dt.float32, kind="ExternalInput")
    w1_d = nc.dram_tensor("w1", [G, H, I], dt.float8e4, kind="ExternalInput")
    b1_d = nc.dram_tensor("b1", [G, I], dt.float32, kind="ExternalInput")
    w2_d = nc.dram_tensor("w2", [G, I, H], dt.float8e4, kind="ExternalInput")
    b2_d = nc.dram_tensor("b2", [G, H], dt.bfloat16, kind="ExternalInput")
    out_d = nc.dram_tensor("out", [B, H], dt.float32, kind="ExternalOutput")

    KH = H // 128   # 6 k-chunks for the H contraction
    KI = I // 128   # 24 k-chunks for the I contraction
    MB = B // 128   # 8 token chunks
    MI = I // 128   # 24 i chunks (fc1 output partitions)

    from concourse.masks import make_identity

    with TileContext(nc) as tc:
        with (
            tc.tile_pool(name="x8p", bufs=1) as x8p,
            tc.tile_pool(name="h1gp", bufs=2) as h1gp,
            tc.tile_pool(name="accp", bufs=1) as accp,
            tc.tile_pool(name="wp", bufs=2) as wp,
            tc.tile_pool(name="constp", bufs=1) as constp,
            tc.tile_pool(name="b1p", bufs=2) as b1p,
            tc.tile_pool(name="wselp", bufs=1) as wselp,
            tc.tile_pool(name="tmpp", bufs=2) as tmpp,
            tc.tile_pool(name="smp", bufs=8) as smp,
            tc.tile_pool(name="routp", bufs=1) as routp,
        ):
            # ---- critical-path DMAs first: x8 + expert-0 fc1 weights ----
            x8 = x8p.tile([128, KH * B], dt.float8e4, name="x8t")
            nc.sync.dma_start(
                out=x8[:].rearrange("p (k b) -> p k b", b=B),
                in_=xt8_d.ap().rearrange("(k p) b -> p k b", p=128),
            )
            w1t0 = wp.tile([128, KH * I], dt.float8e4, name="w1t0", tag="w1")
            for half in range(2):
                io = half * (I // 2)
                nc.sync.dma_start(
                    out=w1t0[:]
                    .rearrange("p (k i) -> p k i", i=I)[:, :, io : io + I // 2],
                    in_=w1_d.ap()[0:1]
                    .rearrange("o (k p) i -> p (o k) i", p=128)[:, :, io : io + I // 2],
                )
            b1t0 = b1p.tile([128, MI], dt.float32, name="b1t0", tag="b1")
            nc.sync.dma_start(
                out=b1t0[:],
                in_=b1_d.ap()[0:1].rearrange("o (m p) -> p (o m)", p=128),
            )

            # ---- routing input DMAs ----
            xtf16 = routp.tile([128, KH * B], dt.bfloat16, name="xtf16t")
            nc.sync.dma_start(
                out=xtf16[:].rearrange("p (k b) -> p k b", b=B),
                in_=xtf16_d.ap().rearrange("(k p) b -> p k b", p=128),
            )
            rfb = routp.tile([128, KH * 27], dt.bfloat16, name="rfbt")
            nc.sync.dma_start(
                out=rfb[:].rearrange("p (k n) -> p k n", n=27),
                in_=rf_d.ap().rearrange("(k p) n -> p k n", p=128),
            )
            rab = routp.tile([128, KH * 27], dt.bfloat16, name="rabt")
            nc.sync.dma_start(
                out=rab[:].rearrange("p (k n) -> p k n", n=27),
                in_=ra_d.ap().rearrange("(k p) n -> p k n", p=128),
            )
            xta16 = routp.tile([128, KH * B], dt.bfloat16, name="xta16t")
            nc.sync.dma_start(
                out=xta16[:].rearrange("p (k b) -> p k b", b=B),
                in_=xta16_d.ap().rearrange("(k p) b -> p k b", p=128),
            )
            bcatT = routp.tile([27, 1], dt.float32, name="bcatTt")
            nc.sync.dma_start(out=bcatT[:], in_=bcat_d.ap())

            acc = accp.tile([128, MB * H], dt.float32, name="acct")
            wsel = wselp.tile([128, MB * G], dt.float32, name="wselt")

            # [ones_bf16 | b2 row j=0 | j=1 | j=2] all on partition 0
            constb = constp.tile([1, 128 + G * H], dt.bfloat16, name="constbt")
            ones_b = constb[0:1, 0:128]
            nc.vector.memset(ones_b, 1.0)
            for j in range(G):
                nc.sync.dma_start(
                    out=constb[0:1, 128 + j * H : 128 + (j + 1) * H],
                    in_=b2_d.ap()[j : j + 1, :],
                )
            ident = constp.tile([32, 32], dt.float32, name="identt")
            make_identity(nc, ident[:])

            # expert-0 fc2 weights right behind the routing inputs
            w2t0 = wp.tile([128, KI * H], dt.float8e4, name="w2t0", tag="w2")
            nc.sync.dma_start(
                out=w2t0[:].rearrange("p (k h) -> p k h", h=H),
                in_=w2_d.ap()[0:1].rearrange("o (k p) h -> p (o k) h", p=128),
            )

            x8v = x8[:].rearrange("p (k b) -> p k b", b=B)

            def emit_fc1(j, w1t, b1t, h1g, ps1):
                w1v = w1t[:].rearrange("p (k i) -> p k i", i=I)
                for m in range(MI):
                    psA = ps1.tile([128, 512], dt.float32, name=f"psA{j}_{m}", tag="ps1t")
                    psB = ps1.tile([128, 512], dt.float32, name=f"psB{j}_{m}", tag="ps1t")
                    for k in range(KH // 2):
                        lhs = w1v[:, 2 * k : 2 * k + 2, m * 128 : (m + 1) * 128]
                        nc.tensor.matmul(
                            psA[:],
                            lhs,
                            x8v[:, 2 * k : 2 * k + 2, 0:512],
                            start=(k == 0),
                            stop=(k == KH // 2 - 1),
                            perf_mode=DR,
                        )
                        nc.tensor.matmul(
                            psB[:],
                            lhs,
                            x8v[:, 2 * k : 2 * k + 2, 512:1024],
                            start=(k == 0),
                            stop=(k == KH // 2 - 1),
                            perf_mode=DR,
                        )
                    nc.scalar.activation(
                        h1g[:, m * B : m * B + 512],
                        psA[:],
                        AF.Gelu,
                        bias=b1t[:, m : m + 1],
                        scale=1.0 / (SX * SW1),
                    )
                    nc.scalar.activation(
                        h1g[:, m * B + 512 : (m + 1) * B],
                        psB[:],
                        AF.Gelu,
                        bias=b1t[:, m : m + 1],
                        scale=1.0 / (SX * SW1),
                    )

            def emit_fc2(j, w2t, h1g, ps2):
                w2v = w2t[:].rearrange("p (k h) -> p k h", h=H)
                h1v = h1g[:].rearrange("p (m b) -> p m b", b=B)
                b2row = constb[0:1, 128 + j * H : 128 + (j + 1) * H]
                for t in range(MB):
                    p2 = ps2.tile([128, 1024], dt.float32, name=f"p2{j}_{t}", tag="p2")
                    nc.tensor.matmul(
                        p2[:, 0:512], ones_b, b2row[:, 0:512], start=True, stop=False
                    )
                    nc.tensor.matmul(
                        p2[:, 512:768], ones_b, b2row[:, 512:768], start=True, stop=False
                    )
                    for k in range(KI // 2):
                        lhs = h1v[:, 2 * k : 2 * k + 2, t * 128 : (t + 1) * 128]
                        nc.tensor.matmul(
                            p2[:, 0:512],
                            lhs,
                            w2v[:, 2 * k : 2 * k + 2, 0:512],
                            start=False,
                            stop=(k == KI // 2 - 1),
                            perf_mode=DR,
                        )
                        nc.tensor.matmul(
                            p2[:, 512:768],
                            lhs,
                            w2v[:, 2 * k : 2 * k + 2, 512:768],
                            start=False,
                            stop=(k == KI // 2 - 1),
                            perf_mode=DR,
                        )
                    # weighted accumulate into acc (wsel carries the 1/SW2):
                    # ScalarE does the p_sel multiply, DVE the accumulate
                    wcol = wsel[:, t * G + j : t * G + j + 1]
                    if j == 0:
                        nc.scalar.mul(acc[:, t * H : (t + 1) * H], p2[:, 0:768], wcol[:])
                    else:
                        tmpc = tmpp.tile([128, H], dt.float32, name=f"tmpc{j}_{t}", tag="tmpc")
                        nc.scalar.mul(tmpc[:], p2[:, 0:768], wcol[:])
                        nc.vector.tensor_add(
                            acc[:, t * H : (t + 1) * H],
                            acc[:, t * H : (t + 1) * H],
                            tmpc[:],
                        )
                    if j == G - 1:
                        # stream this token chunk out as soon as it's final
                        nc.sync.dma_start(
                            out=out_d.ap()[t * 128 : (t + 1) * 128, :],
                            in_=acc[:, t * H : (t + 1) * H],
                        )

            with tc.tile_pool(name="ps1", bufs=4, space="PSUM") as ps1:
                h1g0 = h1gp.tile([128, MI * B], dt.float8e4, name="h1g0", tag="h1g")
                emit_fc1(0, w1t0, b1t0, h1g0, ps1)

                # ---- routing (tensor work hides under fc1 expert 0) ----
                # logitsT[27, B] = [Wcr|Wg]^T x_f + [Wcr_a|0]^T x_a
                expT = routp.tile([27, B], dt.float32, name="expTt")
                with (
                    tc.tile_pool(name="psT", bufs=1, space="PSUM") as psTp,
                    tc.tile_pool(name="psm", bufs=2, space="PSUM") as psmp,
                ):
                    psT = psTp.tile([27, B], dt.float32, name="psTt")
                    for k in range(2 * KH):
                        kk = k % KH
                        rt = rfb if k < KH else rab
                        xt = xtf16 if k < KH else xta16
                        for n in range(2):
                            nc.tensor.matmul(
                                psT[:, n * 512 : (n + 1) * 512],
                                rt[:, kk * 27 : (kk + 1) * 27],
                                xt[:, kk * B + n * 512 : kk * B + (n + 1) * 512],
                                start=(k == 0),
                                stop=(k == 2 * KH - 1),
                            )
                    # exp(logits + bias) in one ACT op (small logits: no max-subtract)
                    nc.scalar.activation(expT[:], psT[:], AF.Exp, bias=bcatT[:])
                    # transpose back to token-major, then pure-DVE softmax tail
                    for m in range(MB):
                        trp = psmp.tile([128, 27], dt.float32, name=f"trp{m}", tag="trp")
                        nc.tensor.transpose(
                            trp[:], expT[:, m * 128 : (m + 1) * 128], ident[0:27, 0:27]
                        )
                        sgv = smp.tile([128, 1], dt.float32, name=f"sg{m}", tag="sg")
                        nc.vector.reduce_sum(
                            sgv[:], trp[:, 24:27], axis=mybir.AxisListType.X
                        )
                        rgv = smp.tile([128, 1], dt.float32, name=f"rg{m}", tag="rg")
                        nc.vector.reciprocal(rgv[:], sgv[:])
                        pgn = smp.tile([128, 3], dt.float32, name=f"pgn{m}", tag="pgn")
                        # fold the 1/SW2 fc2-descale into p_group
                        nc.vector.tensor_scalar(
                            pgn[:],
                            trp[:, 24:27],
                            rgv[:],
                            1.0 / SW2,
                            mybir.AluOpType.mult,
                            mybir.AluOpType.mult,
                        )
                        se3 = smp.tile([128, 3], dt.float32, name=f"se3{m}", tag="se3")
                        nc.vector.reduce_sum(
                            se3[:],
                            trp[:, 0:24].rearrange("p (g e) -> p g e", e=E),
                            axis=mybir.AxisListType.X,
                        )
                        re3 = smp.tile([128, 3], dt.float32, name=f"re3{m}", tag="re3")
                        nc.vector.reciprocal(re3[:], se3[:])
                        pe0 = smp.tile([128, 3], dt.float32, name=f"pe0{m}", tag="pe0")
                        nc.vector.tensor_mul(
                            pe0[:],
                            trp[:, 0:24].rearrange("p (g e) -> p g e", e=E)[:, :, 0],
                            re3[:],
                        )
                        nc.vector.tensor_mul(
                            wsel[:, m * G : (m + 1) * G], pe0[:], pgn[:]
                        )

                # ---- fc2(0) + experts 1,2 ----
                with tc.tile_pool(name="ps2", bufs=2, space="PSUM") as ps2:
                    emit_fc2(0, w2t0, h1g0, ps2)
                    for j in range(1, G):
                        w1t = wp.tile([128, KH * I], dt.float8e4, name=f"w1t{j}", tag="w1")
                        nc.sync.dma_start(
                            out=w1t[:].rearrange("p (k i) -> p k i", i=I),
                            in_=w1_d.ap()[j : j + 1].rearrange(
                                "o (k p) i -> p (o k) i", p=128
                            ),
                        )
                        b1t = b1p.tile([128, MI], dt.float32, name=f"b1t{j}", tag="b1")
                        nc.sync.dma_start(
                            out=b1t[:],
                            in_=b1_d.ap()[j : j + 1].rearrange("o (m p) -> p (o m)", p=128),
                        )
                        w2t = wp.tile([128, KI * H], dt.float8e4, name=f"w2t{j}", tag="w2")
                        nc.sync.dma_start(
                            out=w2t[:].rearrange("p (k h) -> p k h", h=H),
                            in_=w2_d.ap()[j : j + 1].rearrange(
                                "o (k p) h -> p (o k) h", p=128
                            ),
                        )
                        h1g = h1gp.tile(
                            [128, MI * B], dt.float8e4, name=f"h1g{j}", tag="h1g"
                        )
                        emit_fc1(j, w1t, b1t, h1g, ps1)
                        emit_fc2(j, w2t, h1g, ps2)

    nc.compile()
    return nc


def _get_nc():
    global _nc_cache
    if _nc_cache is None:
        _nc_cache = _build_nc()
    return _nc_cache


def _prepare(inputs):
    h_fused = np.asarray(inputs["h_fused"], np.float32)
    h_aspect = np.asarray(inputs["h_aspect"], np.float32)
    Wc = np.asarray(inputs["Wc"], np.float32)
    bc = np.asarray(inputs["bc"], np.float32)
    Wg = np.asarray(inputs["Wg"], np.float32)
    bg = np.asarray(inputs["bg"], np.float32)
    Wr = np.asarray(inputs["Wr"], np.float32)
    br = np.asarray(inputs["br"], np.float32)
    W1 = np.asarray(inputs["W1"], np.float32)
    b1 = np.asarray(inputs["b1"], np.float32)
    W2 = np.asarray(inputs["W2"], np.float32)
    b2 = np.asarray(inputs["b2"], np.float32)

    # fold cond_proj through the expert routers (float64 for the fold)
    Wcr = np.einsum("ch,ghe->cge", Wc.astype(np.float64), Wr.astype(np.float64))
    bcr = np.einsum("h,ghe->ge", bc.astype(np.float64), Wr.astype(np.float64)) + br

    xT = np.ascontiguousarray(h_fused.T)
    xtf16 = xT.astype(BF16)
    xt8 = np.clip(xT * SX, -240.0, 240.0).astype(FP8)
    xta16 = np.ascontiguousarray(h_aspect.T).astype(BF16)

    in_maps = []
    for c in range(NCORES):
        perm = [c] + [e for e in range(E) if e != c]
        Wcr_p = Wcr[:, :, perm]  # [2H, G, E]
        bcr_p = np.asarray(bcr, np.float64)[:, perm]  # [G, E]
        rf = np.concatenate(
            [Wcr_p[:H].reshape(H, G * E), Wg.astype(np.float64)], axis=1
        ).astype(BF16)
        ra = np.concatenate(
            [Wcr_p[H:].reshape(H, G * E), np.zeros((H, G), np.float64)], axis=1
        ).astype(BF16)
        bcat = np.concatenate([bcr_p.reshape(G * E), bg.astype(np.float64)])[
            :, None
        ].astype(np.float32)
        in_maps.append(
            {
                "xtf16": xtf16,
                "xta16": xta16,
                "xt8": xt8,
                "rf": np.ascontiguousarray(rf),
                "ra": np.ascontiguousarray(ra),
                "bcat": np.ascontiguousarray(bcat),
                "w1": np.clip(
                    np.ascontiguousarray(W1[:, c]) * SW1, -240.0, 240.0
                ).astype(FP8),
                "b1": np.ascontiguousarray(b1[:, c]),
                "w2": np.clip(
                    np.ascontiguousarray(W2[:, c]) * SW2, -240.0, 240.0
                ).astype(FP8),
                "b2": (np.ascontiguousarray(b2[:, c]) * SW2).astype(BF16),
            }
        )

    return h_fused, in_maps


def kernel(**inputs):
    from concourse.bass_utils import run_bass_kernel_spmd

    h_fused, in_maps = _prepare(inputs)
    nc = _get_nc()
    res = run_bass_kernel_spmd(nc, in_maps, core_ids=list(range(NCORES)))
    out = h_fused.copy()
    for c in range(NCORES):
        out += res.results[c]["out"]
    return out


def run_traced(**inputs):
    """Profiled run: returns BassKernelResults with exec_time_ns."""
    from concourse.bass_utils import run_bass_kernel_spmd

    h_fused, in_maps = _prepare(inputs)
    nc = _get_nc()
    res = run_bass_kernel_spmd(nc, in_maps, core_ids=list(range(NCORES)), trace=True)
    return res


# revision 7
# speedup vs baseline: 1.1129x; 1.1129x over previous
"""HAGMoE Trainium2 kernel: hierarchical-routed 24-expert MoE, expert-parallel on 8 cores.

Reference computation (B=1024, H=768, I=3072, G=3 groups, E=8 experts/group):
    h_cond  = cat(h_fused, h_aspect) @ Wc + bc
    p_group = softmax(h_fused @ Wg + bg)
    p_exp   = softmax(h_cond @ Wr[g] + br[g])  per group
    h_moe   = sum_{g,e} p_group[:,g] * p_exp[:,g,e] * fc2(gelu(fc1(h_fused)))
    out     = h_fused + h_moe
Sharding: core c owns experts (g, c) for g=0..2 (one expert per group).  The
cond-proj is folded through the expert routers on the host (Wcr = Wc @ Wr), and
within-group expert columns are permuted per core so every core's experts sit at
logit columns {0, 8, 16} -> identical SPMD program, per-core weight data only.
All matmuls (experts AND routing) run in fp8e4 DoubleRow mode (2x PE
throughput, fp32 PSUM accumulate) with host-side scaling to dodge fp8
subnormals; the exp() activation absorbs the routing descale.  The expert-2
combine emits bf16 partials to halve output DMA.  Host gathers:
out = h_fused + sum_c partial_c.
"""

import os
import sys

if "/opt/trn_rl_repo" not in sys.path:
    sys.path.insert(0, "/opt/trn_rl_repo")

import numpy as np
import ml_dtypes

B, H, I, G, E = 1024, 768, 3072, 3, 8
NCORES = 8
BF16 = ml_dtypes.bfloat16
FP8 = ml_dtypes.float8_e4m3

SX = 16.0    # h_fused / h_aspect scale before fp8 cast
SRF = 128.0  # router weight scale before fp8 cast
SW1 = 64.0   # W1 scale before fp8 cast
SW2 = 64.0   # W2 scale before fp8 cast

_nc_cache = None


def _build_nc():
    from concourse import bacc
    import concourse.mybir as mybir
    from concourse.tile import TileContext

    dt = mybir.dt
    AF = mybir.ActivationFunctionType
    DR = mybir.MatmulPerfMode.DoubleRow

    nc = bacc.Bacc("TRN2", target_bir_lowering=False, debug=False, num_devices=NCORES)

    NR = 32  # logit cols (24 expert + 3 group + 5 pad: dual-fp8 ldweights needs even stride)

    # ---- DRAM I/O ----
    xt8_d = nc.dram_tensor("xt8", [H, B], dt.float8e4, kind="ExternalInput")
    xa8_d = nc.dram_tensor("xa8", [H, B], dt.float8e4, kind="ExternalInput")
    rf_d = nc.dram_tensor("rf", [H, NR], dt.float8e4, kind="ExternalInput")
    ra_d = nc.dram_tensor("ra", [H, NR], dt.float8e4, kind="ExternalInput")
    bcat_d = nc.dram_tensor("bcat", [NR, 1], dt.float32, kind="ExternalInput")
    w1_d = nc.dram_tensor("w1", [G, H, I], dt.float8e4, kind="ExternalInput")
    b1_d = nc.dram_tensor("b1", [G, I], dt.float32, kind="ExternalInput")
    w2_d = nc.dram_tensor("w2", [G, I, H], dt.float8e4, kind="ExternalInput")
    b2_d = nc.dram_tensor("b2", [G, H], dt.bfloat16, kind="ExternalInput")
    out_d = nc.dram_tensor("out", [B, H], dt.bfloat16, kind="ExternalOutput")

    KH = H // 128   # 6 k-chunks for the H contraction
    KI = I // 128   # 24 k-chunks for the I contraction
    MB = B // 128   # 8 token chunks
    MI = I // 128   # 24 i chunks (fc1 output partitions)

    from concourse.masks import make_identity

    with TileContext(nc) as tc:
        with (
            tc.tile_pool(name="x8p", bufs=1) as x8p,
            tc.tile_pool(name="h1gp", bufs=2) as h1gp,
            tc.tile_pool(name="accp", bufs=1) as accp,
            tc.tile_pool(name="wp", bufs=2) as wp,
            tc.tile_pool(name="constp", bufs=1) as constp,
            tc.tile_pool(name="b1p", bufs=2) as b1p,
            tc.tile_pool(name="wselp", bufs=1) as wselp,
            tc.tile_pool(name="tmpp", bufs=2) as tmpp,
            tc.tile_pool(name="smp", bufs=8) as smp,
            tc.tile_pool(name="routp", bufs=1) as routp,
        ):
            # ---- DMA issue order tracks the tensor queue's needs ----
            # fc1(0) m0-7 runs first: w1 piece 0 + x8 lead
            w1t0 = wp.tile([128, KH * I], dt.float8e4, name="w1t0", tag="w1")
            w1v0 = w1t0[:].rearrange("p (k i) -> p k i", i=I)
            NP = 3  # w1t0 DMA pieces
            for piece in range(1):
                nc.sync.dma_start(
                    out=w1v0[:, :, 0 : I // NP],
                    in_=w1_d.ap()[0:1]
                    .rearrange("o (k p) i -> p (o k) i", p=128)[:, :, 0 : I // NP],
                )
            x8 = x8p.tile([128, KH * B], dt.float8e4, name="x8t")
            nc.sync.dma_start(
                out=x8[:].rearrange("p (k b) -> p k b", b=B),
                in_=xt8_d.ap().rearrange("(k p) b -> p k b", p=128),
            )
            # routing inputs
            xa8 = routp.tile([128, KH * B], dt.float8e4, name="xa8t")
            nc.sync.dma_start(
                out=xa8[:].rearrange("p (k b) -> p k b", b=B),
                in_=xa8_d.ap().rearrange("(k p) b -> p k b", p=128),
            )
            rfb = routp.tile([128, KH * NR], dt.float8e4, name="rfbt")
            nc.sync.dma_start(
                out=rfb[:].rearrange("p (k n) -> p k n", n=NR),
                in_=rf_d.ap().rearrange("(k p) n -> p k n", p=128),
            )
            rab = routp.tile([128, KH * NR], dt.float8e4, name="rabt")
            nc.sync.dma_start(
                out=rab[:].rearrange("p (k n) -> p k n", n=NR),
                in_=ra_d.ap().rearrange("(k p) n -> p k n", p=128),
            )
            bcatT = routp.tile([NR, 1], dt.float32, name="bcatTt")
            nc.sync.dma_start(out=bcatT[:], in_=bcat_d.ap())
            b1t0 = b1p.tile([128, MI], dt.float32, name="b1t0", tag="b1")
            nc.sync.dma_start(
                out=b1t0[:],
                in_=b1_d.ap()[0:1].rearrange("o (m p) -> p (o m)", p=128),
            )
            # rest of w1(0)
            for piece in range(1, NP):
                io = piece * (I // NP)
                nc.sync.dma_start(
                    out=w1v0[:, :, io : io + I // NP],
                    in_=w1_d.ap()[0:1]
                    .rearrange("o (k p) i -> p (o k) i", p=128)[:, :, io : io + I // NP],
                )
            # b2 replicated across partitions (for the DVE bias path)
            b2repl = constp.tile([128, G * H], dt.bfloat16, name="b2replt")
            nc.sync.dma_start(
                out=b2repl[:],
                in_=b2_d.ap().rearrange("g h -> () (g h)").broadcast_to([128, G * H]),
            )
            w2t0 = wp.tile([128, KI * H], dt.float8e4, name="w2t0", tag="w2")
            nc.sync.dma_start(
                out=w2t0[:].rearrange("p (k h) -> p k h", h=H),
                in_=w2_d.ap()[0:1].rearrange("o (k p) h -> p (o k) h", p=128),
            )

            acc = accp.tile([128, MB * H], dt.float32, name="acct")
            accb = accp.tile([128, MB * H], dt.bfloat16, name="accbt")
            wsel = wselp.tile([128, MB * G], dt.float32, name="wselt")
            ident = constp.tile([32, 32], dt.float32, name="identt")
            make_identity(nc, ident[:])

            x8v = x8[:].rearrange("p (k b) -> p k b", b=B)
            xa8v = xa8[:].rearrange("p (k b) -> p k b", b=B)
            rfv = rfb[:].rearrange("p (k n) -> p k n", n=NR)
            rav = rab[:].rearrange("p (k n) -> p k n", n=NR)

            def emit_fc1(j, w1v, b1t, h1g, ps1, m_range):
                for m in m_range:
                    psA = ps1.tile([128, 512], dt.float32, name=f"psA{j}_{m}", tag="ps1t")
                    psB = ps1.tile([128, 512], dt.float32, name=f"psB{j}_{m}", tag="ps1t")
                    for k in range(KH // 2):
                        lhs = w1v[:, 2 * k : 2 * k + 2, m * 128 : (m + 1) * 128]
                        nc.tensor.matmul(
                            psA[:],
                            lhs,
                            x8v[:, 2 * k : 2 * k + 2, 0:512],
                            start=(k == 0),
                            stop=(k == KH // 2 - 1),
                            perf_mode=DR,
                        )
                        nc.tensor.matmul(
                            psB[:],
                            lhs,
                            x8v[:, 2 * k : 2 * k + 2, 512:1024],
                            start=(k == 0),
                            stop=(k == KH // 2 - 1),
                            perf_mode=DR,
                        )
                    nc.scalar.activation(
                        h1g[:, m * B : m * B + 512],
                        psA[:],
                        AF.Gelu,
                        bias=b1t[:, m : m + 1],
                        scale=1.0 / (SX * SW1),
                    )
                    nc.scalar.activation(
                        h1g[:, m * B + 512 : (m + 1) * B],
                        psB[:],
                        AF.Gelu,
                        bias=b1t[:, m : m + 1],
                        scale=1.0 / (SX * SW1),
                    )

            def emit_fc2(j, w2t, h1g, ps2):
                w2v = w2t[:].rearrange("p (k h) -> p k h", h=H)
                h1v = h1g[:].rearrange("p (m b) -> p m b", b=B)
                for t in range(MB):
                    p2 = ps2.tile([128, 1024], dt.float32, name=f"p2{j}_{t}", tag="p2")
                    for k in range(KI // 2):
                        lhs = h1v[:, 2 * k : 2 * k + 2, t * 128 : (t + 1) * 128]
                        nc.tensor.matmul(
                            p2[:, 0:512],
                            lhs,
                            w2v[:, 2 * k : 2 * k + 2, 0:512],
                            start=(k == 0),
                            stop=(k == KI // 2 - 1),
                            perf_mode=DR,
                        )
                        nc.tensor.matmul(
                            p2[:, 512:768],
                            lhs,
                            w2v[:, 2 * k : 2 * k + 2, 512:768],
                            start=(k == 0),
                            stop=(k == KI // 2 - 1),
                            perf_mode=DR,
                        )
                    # weighted accumulate into acc (wsel carries the 1/SW2):
                    # ScalarE does the p_sel multiply, DVE the accumulate.
                    # acc[t] was pre-initialized with the b2 bias term.
                    wcol = wsel[:, t * G + j : t * G + j + 1]
                    tmpc = tmpp.tile([128, H], dt.float32, name=f"tmpc{j}_{t}", tag="tmpc")
                    nc.scalar.mul(tmpc[:], p2[:, 0:768], wcol[:])
                    if j < G - 1:
                        nc.vector.tensor_add(
                            acc[:, t * H : (t + 1) * H],
                            acc[:, t * H : (t + 1) * H],
                            tmpc[:],
                        )
                    else:
                        # final expert: emit bf16 partial and stream it out
                        nc.vector.tensor_add(
                            accb[:, t * H : (t + 1) * H],
                            acc[:, t * H : (t + 1) * H],
                            tmpc[:],
                        )
                        nc.sync.dma_start(
                            out=out_d.ap()[t * 128 : (t + 1) * 128, :],
                            in_=accb[:, t * H : (t + 1) * H],
                        )

            with tc.tile_pool(name="ps1", bufs=4, space="PSUM") as ps1:
                h1g0 = h1gp.tile([128, MI * B], dt.float8e4, name="h1g0", tag="h1g")
                expT = routp.tile([NR, B], dt.float32, name="expTt")

                with (
                    tc.tile_pool(name="psT", bufs=1, space="PSUM") as psTp,
                    tc.tile_pool(name="psm", bufs=2, space="PSUM") as psmp,
                ):
                    # fc1(0) m0-7 leads; routing matmuls slot in once their
                    # (smaller) inputs have landed
                    emit_fc1(0, w1v0, b1t0, h1g0, ps1, range(0, 8))

                    # routing: logitsT[27, B] in fp8 DoubleRow, scale absorbed
                    # by the exp() activation
                    psT = psTp.tile([NR, B], dt.float32, name="psTt")
                    for k in range(KH // 2):
                        for n in range(2):
                            nc.tensor.matmul(
                                psT[:, n * 512 : (n + 1) * 512],
                                rfv[:, 2 * k : 2 * k + 2, :],
                                x8v[:, 2 * k : 2 * k + 2, n * 512 : (n + 1) * 512],
                                start=(k == 0),
                                stop=False,
                                perf_mode=DR,
                            )
                    for k in range(KH // 2):
                        for n in range(2):
                            nc.tensor.matmul(
                                psT[:, n * 512 : (n + 1) * 512],
                                rav[:, 2 * k : 2 * k + 2, :],
                                xa8v[:, 2 * k : 2 * k + 2, n * 512 : (n + 1) * 512],
                                start=False,
                                stop=(k == KH // 2 - 1),
                                perf_mode=DR,
                            )
                    # exp(logits + bias) in one ACT op (small logits: no
                    # max-subtract); 1/2048 descale folded into the ACT
                    nc.scalar.activation(
                        expT[:], psT[:], AF.Exp, bias=bcatT[:], scale=1.0 / (SX * SRF)
                    )

                    emit_fc1(0, w1v0, b1t0, h1g0, ps1, range(8, MI))

                    # transpose expT to token-major, pure-DVE softmax tail
                    for m in range(MB):
                        trp = psmp.tile([128, NR], dt.float32, name=f"trp{m}", tag="trp")
                        nc.tensor.transpose(
                            trp[:], expT[:, m * 128 : (m + 1) * 128], ident[0:NR, 0:NR]
                        )
                        sgv = smp.tile([128, 1], dt.float32, name=f"sg{m}", tag="sg")
                        nc.vector.reduce_sum(
                            sgv[:], trp[:, 24:27], axis=mybir.AxisListType.X
                        )
                        rgv = smp.tile([128, 1], dt.float32, name=f"rg{m}", tag="rg")
                        nc.vector.reciprocal(rgv[:], sgv[:])
                        pgn = smp.tile([128, 3], dt.float32, name=f"pgn{m}", tag="pgn")
                        # fold the 1/SW2 fc2-descale into p_group
                        nc.vector.tensor_scalar(
                            pgn[:],
                            trp[:, 24:27],
                            rgv[:],
                            1.0 / SW2,
                            mybir.AluOpType.mult,
                            mybir.AluOpType.mult,
                        )
                        se3 = smp.tile([128, 3], dt.float32, name=f"se3{m}", tag="se3")
                        nc.vector.reduce_sum(
                            se3[:],
                            trp[:, 0:24].rearrange("p (g e) -> p g e", e=E),
                            axis=mybir.AxisListType.X,
                        )
                        re3 = smp.tile([128, 3], dt.float32, name=f"re3{m}", tag="re3")
                        nc.vector.reciprocal(re3[:], se3[:])
                        pe0 = smp.tile([128, 3], dt.float32, name=f"pe0{m}", tag="pe0")
                        nc.vector.tensor_mul(
                            pe0[:],
                            trp[:, 0:24].rearrange("p (g e) -> p g e", e=E)[:, :, 0],
                            re3[:],
                        )
                        nc.vector.tensor_mul(
                            wsel[:, m * G : (m + 1) * G], pe0[:], pgn[:]
                        )
                        # acc[t] bias init: sum_j wselB[t,j] * (SW2*b2[j,:]) / SW2
                        # (b2repl holds SW2*b2, wsel carries 1/SW2 -> use wsel)
                        bt = tmpp.tile([128, H], dt.float32, name=f"bt{m}", tag="tmpc")
                        nc.vector.tensor_scalar_mul(
                            acc[:, m * H : (m + 1) * H],
                            b2repl[:, 0:H],
                            wsel[:, m * G : m * G + 1],
                        )
                        for j in range(1, G):
                            nc.vector.tensor_scalar_mul(
                                bt[:],
                                b2repl[:, j * H : (j + 1) * H],
                                wsel[:, m * G + j : m * G + j + 1],
                            )
                            nc.vector.tensor_add(
                                acc[:, m * H : (m + 1) * H],
                                acc[:, m * H : (m + 1) * H],
                                bt[:],
                            )

                # ---- fc2(0) + experts 1,2 ----
                with tc.tile_pool(name="ps2", bufs=2, space="PSUM") as ps2:
                    emit_fc2(0, w2t0, h1g0, ps2)
                    for j in range(1, G):
                        w1t = wp.tile([128, KH * I], dt.float8e4, name=f"w1t{j}", tag="w1")
                        nc.sync.dma_start(
                            out=w1t[:].rearrange("p (k i) -> p k i", i=I),
                            in_=w1_d.ap()[j : j + 1].rearrange(
                                "o (k p) i -> p (o k) i", p=128
                            ),
                        )
                        b1t = b1p.tile([128, MI], dt.float32, name=f"b1t{j}", tag="b1")
                        nc.sync.dma_start(
                            out=b1t[:],
                            in_=b1_d.ap()[j : j + 1].rearrange("o (m p) -> p (o m)", p=128),
                        )
                        w2t = wp.tile([128, KI * H], dt.float8e4, name=f"w2t{j}", tag="w2")
                        nc.sync.dma_start(
                            out=w2t[:].rearrange("p (k h) -> p k h", h=H),
                            in_=w2_d.ap()[j : j + 1].rearrange(
                                "o (k p) h -> p (o k) h", p=128
                            ),
                        )
                        h1g = h1gp.tile(
                            [128, MI * B], dt.float8e4, name=f"h1g{j}", tag="h1g"
                        )
                        w1v = w1t[:].rearrange("p (k i) -> p k i", i=I)
                        emit_fc1(j, w1v, b1t, h1g, ps1, range(MI))
                        emit_fc2(j, w2t, h1g, ps2)

    nc.compile()
    return nc


def _get_nc():
    global _nc_cache
    if _nc_cache is None:
        _nc_cache = _build_nc()
    return _nc_cache


def _prepare(inputs):
    h_fused = np.asarray(inputs["h_fused"], np.float32)
    h_aspect = np.asarray(inputs["h_aspect"], np.float32)
    Wc = np.asarray(inputs["Wc"], np.float32)
    bc = np.asarray(inputs["bc"], np.float32)
    Wg = np.asarray(inputs["Wg"], np.float32)
    bg = np.asarray(inputs["bg"], np.float32)
    Wr = np.asarray(inputs["Wr"], np.float32)
    br = np.asarray(inputs["br"], np.float32)
    W1 = np.asarray(inputs["W1"], np.float32)
    b1 = np.asarray(inputs["b1"], np.float32)
    W2 = np.asarray(inputs["W2"], np.float32)
    b2 = np.asarray(inputs["b2"], np.float32)

    def q8(x, s):
        return np.clip(np.asarray(x, np.float64) * s, -240.0, 240.0).astype(FP8)

    # fold cond_proj through the expert routers (float64 for the fold)
    Wcr = np.einsum("ch,ghe->cge", Wc.astype(np.float64), Wr.astype(np.float64))
    bcr = np.einsum("h,ghe->ge", bc.astype(np.float64), Wr.astype(np.float64)) + br

    xT = np.ascontiguousarray(h_fused.T)
    xt8 = q8(xT, SX)
    xa8 = q8(np.ascontiguousarray(h_aspect.T), SX)

    in_maps = []
    for c in range(NCORES):
        perm = [c] + [e for e in range(E) if e != c]
        Wcr_p = Wcr[:, :, perm]  # [2H, G, E]
        bcr_p = np.asarray(bcr, np.float64)[:, perm]  # [G, E]
        rf = np.concatenate(
            [Wcr_p[:H].reshape(H, G * E), Wg.astype(np.float64),
             np.zeros((H, 5), np.float64)], axis=1
        )
        ra = np.concatenate(
            [Wcr_p[H:].reshape(H, G * E), np.zeros((H, G + 5), np.float64)], axis=1
        )
        bcat = np.concatenate(
            [bcr_p.reshape(G * E), bg.astype(np.float64), np.zeros(5)]
        )[:, None].astype(np.float32)
        in_maps.append(
            {
                "xt8": xt8,
                "xa8": xa8,
                "rf": q8(np.ascontiguousarray(rf), SRF),
                "ra": q8(np.ascontiguousarray(ra), SRF),
                "bcat": np.ascontiguousarray(bcat),
                "w1": q8(np.ascontiguousarray(W1[:, c]), SW1),
                "b1": np.ascontiguousarray(b1[:, c]),
                "w2": q8(np.ascontiguousarray(W2[:, c]), SW2),
                "b2": (np.ascontiguousarray(b2[:, c]) * SW2).astype(BF16),
            }
        )

    return h_fused, in_maps


def kernel(**inputs):
    from concourse.bass_utils import run_bass_kernel_spmd

    h_fused, in_maps = _prepare(inputs)
    nc = _get_nc()
    res = run_bass_kernel_spmd(nc, in_maps, core_ids=list(range(NCORES)))
    out = h_fused.copy()
    for c in range(NCORES):
        out += np.asarray(res.results[c]["out"], np.float32)
    return out


def run_traced(**inputs):
    """Profiled run: returns BassKernelResults with exec_time_ns."""
    from concourse.bass_utils import run_bass_kernel_spmd

    h_fused, in_maps = _prepare(inputs)
    nc = _get_nc()
    res = run_bass_kernel_spmd(nc, in_maps, core_ids=list(range(NCORES)), trace=True)
    return res
